# revision 44
# baseline (speedup 1.0000x reference)
"""MoE (top-2 of 8 experts, SwiGLU FFN) on 8 Trainium2 NeuronCores.

Strategy: expert-parallel. Routing (gate matmul + top-2 + softmax) is done
on the host in numpy; tokens are gathered per expert, padded to a common
capacity C, and each core runs the full SwiGLU FFN for one expert's tokens.
The host scatters the per-core outputs back with the gate weights.

Matmuls use fp8(e4m3) DoubleRow mode (0.5 PE cycles/row, 2 contraction
slices per instruction) with a 3-term hi/lo decomposition so accuracy
stays at bf16 level while the PE runs 1.33x faster than bf16:

    W @ x ~= W_hi@x_hi + W_hi@x_lo + W_lo@x_hi      (all e4m3)

where W_hi = q8(SW*W), W_lo = q8(SW*W - W_hi) (SW=32 keeps the small
weights out of e4m3's subnormal range), x_hi = q8(x), x_lo = q8(x - x_hi).
All three terms share the PSUM scale SW so they accumulate in one group;
the 1/SW descale is folded into the silu activation's input scale. The
activations are split the same way on device (act scale SA=8; the hi part
is captured by an Act-engine Copy with scale SA/SW, the lo part by a DVE
scalar_tensor_tensor; the last h-pair keeps the whole chain on DVE to cut
cross-engine hops off stage 2's critical path), and stage 2's output
descale 1/(SW*SA) is folded into its PSUM->SBUF copy, emitted in bf16.

Device layouts (per core, pre-tiled on host so every DMA is contiguous):
  xth/xtl   [128, KD, C]      f8e4  xT hi/lo tiles
  w0h/w0l   [128, KH, KD,128] f8e4  w0[e].T h-tile-major hi/lo
  w1h/w1l   [128, KH, KD,128] f8e4
  w2h/w2l   [128, KH, D]      f8e4  w2[e].T tiles (h on part, d on free)
  b0s/b1t   [128, KH]         fp32  b0*SW and b1 per-partition biases
  out       [128, KD, C]      bf16  transposed: out[p,k,c]=ffn_out[c,k*128+p]

Stage 1 computes h0^T/h1^T (h on partitions, tokens on free dim) so that
stage 2 can contract over h without any on-device transpose; stage 2 keeps
d on partitions so its moving dim is the (possibly ragged) token chunk.
"""

import os

import numpy as np
import ml_dtypes

# The tunneled trn2 cores occasionally come up wedged from a prior process;
# asking the runtime to reset cores on init recovers them.
os.environ.setdefault("NEURON_RT_RESET_CORES", "1")

E, TOPK, D, H = 8, 2, 1024, 2048
NCORES = 8
P = 128
KD = D // P   # 8 d-tiles
KH = H // P   # 16 h-tiles
BF16 = ml_dtypes.bfloat16
F8 = ml_dtypes.float8_e4m3
SW = 32.0     # weight quantization scale
SA = 8.0      # activation quantization scale

_build_cache: dict = {}
_ACT_SILU = True  # CoreSim lacks Silu; tests may flip this to Tanh
_N_WARM = 4       # PE warm-up matmuls (cover p-state ramp + first DMAs)
_TAIL = 164       # last-chunk width (short kernel drain, still PE-bound)
_LASTP = 24       # final out piece width (bounds the kernel drain chain)


def _chunk_plan(C: int):
    """Token-chunk widths. A full 512 chunk FIRST so the PE consumes h-tiles
    slower than the serialized DMA stream delivers weight pieces; a small
    TAIL chunk last so the kernel drain (last copies + out DMA) is short.
    Middle chunks must stay >= ~308 so the Act/DVE epilogue per h-tile
    doesn't become the pipeline bottleneck."""
    TAIL = _TAIL
    MINMID = 308
    if C <= 512:
        return [C]
    if C <= 512 + TAIL:
        return [512, C - 512]
    rem = C - TAIL
    n512 = rem // 512
    mid = rem - n512 * 512
    mids = []
    if mid and mid < MINMID and n512 >= 1:
        n512 -= 1
        a = mid + 512
        mids = [a - a // 2, a // 2] if a > 512 else [a]
    elif mid:
        mids = [mid]
    return [512] * n512 + mids + [TAIL]


def _build_bass(C: int, repeat: int = 1):
    """Build the single-core SPMD Bass program for capacity C."""
    import concourse.bacc as bacc
    import concourse.mybir as mybir
    from concourse import tile

    fp32 = mybir.dt.float32
    bf16 = mybir.dt.bfloat16
    f8e4 = mybir.dt.float8e4
    AF = mybir.ActivationFunctionType
    ALU = mybir.AluOpType
    DR = mybir.MatmulPerfMode.DoubleRow

    # token chunks (free dim of the matmuls); any width <= 512. A smaller
    # first chunk lets the PE start while the bulk DMA is still in flight.
    chunks = _chunk_plan(C)

    nc = bacc.Bacc("TRN2", target_bir_lowering=False)
    xth_d = nc.dram_tensor("xth", [P, KD, C], f8e4, kind="ExternalInput")
    xtl_d = nc.dram_tensor("xtl", [P, KD, C], f8e4, kind="ExternalInput")
    # w0/w1 are h-tile-major so any h-tile piece is a fully contiguous DMA
    w0h_d = nc.dram_tensor("w0h", [P, KH, KD, P], f8e4, kind="ExternalInput")
    w0l_d = nc.dram_tensor("w0l", [P, KH, KD, P], f8e4, kind="ExternalInput")
    w1h_d = nc.dram_tensor("w1h", [P, KH, KD, P], f8e4, kind="ExternalInput")
    w1l_d = nc.dram_tensor("w1l", [P, KH, KD, P], f8e4, kind="ExternalInput")
    w2h_d = nc.dram_tensor("w2h", [P, KH, D], f8e4, kind="ExternalInput")
    w2l_d = nc.dram_tensor("w2l", [P, KH, D], f8e4, kind="ExternalInput")
    b0s_d = nc.dram_tensor("b0s", [P, KH], fp32, kind="ExternalInput")
    b1t_d = nc.dram_tensor("b1t", [P, KH], fp32, kind="ExternalInput")
    # out is produced transposed: out_t[p, k, c] = ffn_out[c, k*128+p]
    out_d = nc.dram_tensor("out", [P, KD, C], bf16, kind="ExternalOutput")

    with tile.TileContext(nc) as tc:
        with (
            tc.tile_pool(name="wconst", bufs=1) as wpool,
            tc.tile_pool(name="xtp", bufs=1) as xpool,
            tc.tile_pool(name="act", bufs=2) as apool,
            tc.tile_pool(name="sil", bufs=4) as spool,
            tc.tile_pool(name="af32", bufs=4) as fpool,
            tc.tile_pool(name="osb", bufs=4) as opool,
            tc.tile_pool(name="ps0", bufs=2, space="PSUM") as pp0,
            tc.tile_pool(name="ps1", bufs=2, space="PSUM") as pp1,
            tc.tile_pool(name="pso", bufs=3, space="PSUM") as ppo,
            tc.tile_pool(name="warm", bufs=1, space="PSUM") as wppool,
        ):
            w0h_sb = wpool.tile([P, KH, KD, P], f8e4, tag="w0h")
            w0l_sb = wpool.tile([P, KH, KD, P], f8e4, tag="w0l")
            w1h_sb = wpool.tile([P, KH, KD, P], f8e4, tag="w1h")
            w1l_sb = wpool.tile([P, KH, KD, P], f8e4, tag="w1l")
            w2h_sb = wpool.tile([P, KH, D], f8e4, tag="w2h")
            w2l_sb = wpool.tile([P, KH, D], f8e4, tag="w2l")
            b0s_sb = wpool.tile([P, KH], fp32, tag="b0s")
            b1t_sb = wpool.tile([P, KH], fp32, tag="b1t")
            # Warm the PE (HAM clock gate / p-state ramp) with dummy matmuls
            # on a zeroed tile while the first weight/token DMAs are in
            # flight — the ramp to full clock happens before real work.
            z_sb = wpool.tile([P, P], bf16, tag="warmz")
            nc.scalar.memzero(z_sb[:])
            zp = wppool.tile([P, P], fp32, tag="warmp")
            n_warm = _N_WARM if C >= 768 else 16
            for _ in range(n_warm):
                nc.tensor.matmul(zp[:], z_sb[:], z_sb[:], start=True, stop=True)

            # xt streams per chunk through the pool (one buf per chunk so xt
            # DMAs never wait on tile reuse and can't block the DGE queue).
            xt_tiles = []
            # Load order matters: transfers drain serially in issue order, so
            # deliver exactly in first-use order: (w1h,xh,w1l,w0h,w0l,xl) for
            # h-piece 0, then the remaining h-pieces, then w2 d-halves, then
            # the later token chunks.
            hpieces = [(0, 2), (2, 2), (4, 2), (6, 2), (8, 4), (12, 4)]
            j0_, jw_ = hpieces[0]
            js0 = slice(j0_, j0_ + jw_)
            nc.sync.dma_start(w1h_sb[:, js0], w1h_d[:, js0])
            xth0 = xpool.tile([P, KD, chunks[0]], f8e4, tag="xth0")
            xtl0 = xpool.tile([P, KD, chunks[0]], f8e4, tag="xtl0")
            hd = KD // 2
            nc.sync.dma_start(xth0[:, 0:hd], xth_d[:, 0:hd, 0:chunks[0]])
            nc.sync.dma_start(xth0[:, hd:], xth_d[:, hd:, 0:chunks[0]])
            nc.sync.dma_start(w1l_sb[:, js0], w1l_d[:, js0])
            nc.sync.dma_start(xtl0[:, 0:hd], xtl_d[:, 0:hd, 0:chunks[0]])
            nc.sync.dma_start(xtl0[:, hd:], xtl_d[:, hd:, 0:chunks[0]])
            nc.sync.dma_start(b1t_sb[:], b1t_d[:])
            nc.sync.dma_start(w0h_sb[:, js0], w0h_d[:, js0])
            nc.sync.dma_start(w0l_sb[:, js0], w0l_d[:, js0])
            nc.sync.dma_start(b0s_sb[:], b0s_d[:])
            xt_tiles.append((xth0, xtl0))
            for j0_, jw_ in hpieces[1:]:
                js_ = slice(j0_, j0_ + jw_)
                nc.sync.dma_start(w1h_sb[:, js_], w1h_d[:, js_])
                nc.sync.dma_start(w1l_sb[:, js_], w1l_d[:, js_])
                nc.sync.dma_start(w0h_sb[:, js_], w0h_d[:, js_])
                nc.sync.dma_start(w0l_sb[:, js_], w0l_d[:, js_])
            # w2 streamed in d-halves (512B lines keep full DMA throughput),
            # ordered hi.a, lo.a, hi.b, lo.b to match stage-2 d-tile order.
            nc.sync.dma_start(w2h_sb[:, :, 0:512], w2h_d[:, :, 0:512])
            nc.sync.dma_start(w2l_sb[:, :, 0:512], w2l_d[:, :, 0:512])
            nc.sync.dma_start(w2h_sb[:, :, 512:D], w2h_d[:, :, 512:D])
            nc.sync.dma_start(w2l_sb[:, :, 512:D], w2l_d[:, :, 512:D])
            cpos = chunks[0]
            for ci_, tcw_ in enumerate(chunks[1:]):
                xth_ch = xpool.tile([P, KD, tcw_], f8e4, tag=f"xth{ci_ + 1}")
                xtl_ch = xpool.tile([P, KD, tcw_], f8e4, tag=f"xtl{ci_ + 1}")
                nc.sync.dma_start(xth_ch[:], xth_d[:, :, cpos:cpos + tcw_])
                nc.sync.dma_start(xtl_ch[:], xtl_d[:, :, cpos:cpos + tcw_])
                xt_tiles.append((xth_ch, xtl_ch))
                cpos += tcw_

            def _s1_part(ps, w_sb, ht, x_sb, start, stop):
                """4 DoubleRow matmuls: one term of a stage-1 group."""
                for j in range(KD // 2):
                    nc.tensor.matmul(
                        ps[:],
                        w_sb[:, ht, 2 * j:2 * j + 2, :],
                        x_sb[:, 2 * j:2 * j + 2, :],
                        start=start and j == 0,
                        stop=stop and j == KD // 2 - 1,
                        perf_mode=DR,
                    )

            def _s1_group(ps, wh_sb, wl_sb, ht, xh, xl, skip_first=False):
                """12 DoubleRow matmuls: Wh@xh + Wl@xh + Wh@xl (scale SW).
                With skip_first the Wh@xh term was already emitted early."""
                if not skip_first:
                    _s1_part(ps, wh_sb, ht, xh, True, False)
                _s1_part(ps, wl_sb, ht, xh, False, False)
                _s1_part(ps, wh_sb, ht, xl, False, True)

            def _body():
                c0 = 0
                for ci, tcw in enumerate(chunks):
                    xh_sb, xl_sb = xt_tiles[ci]
                    # per-h-pair act tiles: stage 2's pair-j matmul depends
                    # only on pair j's epilogue, so it can start while the
                    # last h-tiles' act split is still in flight.
                    ah_sb = [apool.tile([P, 2, tcw], f8e4, tag=f"ah{j}",
                                        name=f"ah{j}_{ci}")
                             for j in range(KH // 2)]
                    al_sb = [apool.tile([P, 2, tcw], f8e4, tag=f"al{j}",
                                        name=f"al{j}_{ci}")
                             for j in range(KH // 2)]
                    # For the very first chunk, emit the Wh@xh term of the
                    # first two h-tiles' groups up front: it only needs the
                    # first weight piece + x_hi, so the PE has useful work
                    # while w_lo / x_lo are still streaming in.
                    pre = {}
                    if ci == 0:
                        for ht in (0, 1):
                            ps1 = pp1.tile([P, tcw], fp32, tag="ps1")
                            _s1_part(ps1, w1h_sb, ht, xh_sb, True, False)
                            ps0 = pp0.tile([P, tcw], fp32, tag="ps0")
                            _s1_part(ps0, w0h_sb, ht, xh_sb, True, False)
                            pre[ht] = (ps1, ps0)
                    for ht in range(KH):
                        if ht in pre:
                            ps1, ps0 = pre[ht]
                            _s1_group(ps1, w1h_sb, w1l_sb, ht, xh_sb, xl_sb,
                                      skip_first=True)
                            _s1_group(ps0, w0h_sb, w0l_sb, ht, xh_sb, xl_sb,
                                      skip_first=True)
                        else:
                            ps1 = pp1.tile([P, tcw], fp32, tag="ps1")
                            _s1_group(ps1, w1h_sb, w1l_sb, ht, xh_sb, xl_sb)
                            ps0 = pp0.tile([P, tcw], fp32, tag="ps0")
                            _s1_group(ps0, w0h_sb, w0l_sb, ht, xh_sb, xl_sb)
                        sil = spool.tile([P, tcw], fp32, tag="sil")
                        af = AF.Silu if _ACT_SILU else AF.Tanh
                        # sil = silu(h1 + b1) with h1 = ps1/SW
                        nc.scalar.activation(
                            sil[:], ps1[:], af,
                            bias=b1t_sb[:, ht:ht + 1], scale=1.0 / SW,
                        )
                        # af32 = (ps0 + SW*b0) * sil = SW * act
                        last_pair = ht >= KH - 2
                        af32 = fpool.tile([P, tcw], fp32, tag="af32")
                        nc.vector.scalar_tensor_tensor(
                            af32[:], ps0[:], b0s_sb[:, ht:ht + 1], sil[:],
                            ALU.add, ALU.mult,
                        )
                        # act_hi = q8(SA * act), scale SA/SW
                        ah_t = ah_sb[ht // 2][:, ht % 2, :]
                        if last_pair:
                            # keep the whole chain on DVE: no cross-engine
                            # sem hops on the path stage 2's last pair waits on
                            nc.vector.tensor_scalar(
                                ah_t, af32[:], SA / SW, None, ALU.mult,
                            )
                        else:
                            nc.scalar.activation(
                                ah_t, af32[:], AF.Copy, scale=SA / SW,
                            )
                        # act_lo = q8(SA*act - act_hi)
                        nc.vector.scalar_tensor_tensor(
                            al_sb[ht // 2][:, ht % 2, :], af32[:], SA / SW,
                            ah_t, ALU.mult, ALU.subtract,
                        )
                    # stage 2 transposed: out_t[d-tile] = w2T_tile.T @ act
                    last_chunk = c0 + tcw == C
                    for dk in range(KD):
                        # split the very last group into token-halves so the
                        # first half's copy+DMA overlaps the second half's
                        # matmuls (shorter kernel tail)
                        if last_chunk and dk == KD - 1 and tcw >= 128:
                            halves = [(0, tcw - _LASTP), (tcw - _LASTP, _LASTP)]
                        else:
                            halves = [(0, tcw)]
                        ds = slice(dk * P, (dk + 1) * P)
                        for f0, fw in halves:
                            pso = ppo.tile([P, fw], fp32, tag="pso")
                            terms = [(w2h_sb, ah_sb), (w2l_sb, ah_sb),
                                     (w2h_sb, al_sb)]
                            # j-major so pair j's act tiles are needed as
                            # late as possible: the last h-pair's epilogue
                            # (still in flight right after stage 1) is only
                            # read by matmuls 21..23 of the group.
                            n = len(terms) * (KH // 2)
                            i = 0
                            for j in range(KH // 2):
                                for w_sb, a_sb in terms:
                                    nc.tensor.matmul(
                                        pso[:],
                                        w_sb[:, 2 * j:2 * j + 2, ds],
                                        a_sb[j][:, :, f0:f0 + fw],
                                        start=(i == 0),
                                        stop=(i == n - 1),
                                        perf_mode=DR,
                                    )
                                    i += 1
                            o_sb = opool.tile([P, fw], bf16, tag="osb")
                            # descale 1/(SW*SA) folded into the PSUM->SBUF copy
                            nc.scalar.activation(
                                o_sb[:], pso[:], AF.Copy, scale=1.0 / (SW * SA),
                            )
                            nc.sync.dma_start(
                                out_d[:, dk, c0 + f0:c0 + f0 + fw], o_sb[:]
                            )
                    c0 += tcw

            if repeat == 1:
                _body()
            else:
                # hardware loop: constant program size for any repeat count
                # (used only for wall-clock benchmarking of the body)
                with tc.For_i(0, repeat, 1):
                    _body()
    nc.compile()
    return nc


def _get_bass(C: int, repeat: int = 1):
    key = (C, repeat)
    if key not in _build_cache:
        _build_cache[key] = _build_bass(C, repeat)
    return _build_cache[key]


_runner_cache: dict = {}


def _get_runner(C: int, repeat: int = 1):
    """Compile the SPMD program once and return a reusable launcher.

    Mirrors concourse.bass2jax.run_bass_via_pjrt but memoizes the jitted
    executable so repeated kernel() calls don't recompile the NEFF.
    """
    key = (C, repeat)
    if key in _runner_cache:
        return _runner_cache[key]

    import jax
    from jax.experimental.shard_map import shard_map
    from jax.sharding import Mesh, PartitionSpec
    import concourse.mybir as mybir
    from concourse import bass2jax

    nc = _get_bass(C, repeat)
    bass2jax.install_neuronx_cc_hook()
    partition_name = nc.partition_id_tensor.name if nc.partition_id_tensor else None

    in_names: list = []
    out_names: list = []
    out_avals: list = []
    out_shapes: list = []
    for alloc in nc.m.functions[0].allocations:
        if not isinstance(alloc, mybir.MemoryLocationSet):
            continue
        name = alloc.memorylocations[0].name
        if alloc.kind == "ExternalInput":
            if name != partition_name:
                in_names.append(name)
        elif alloc.kind == "ExternalOutput":
            shape = tuple(alloc.tensor_shape)
            dtype = mybir.dt.np(alloc.dtype)
            out_names.append(name)
            out_avals.append(jax.core.ShapedArray(shape, dtype))
            out_shapes.append((shape, dtype))
    n_params = len(in_names)
    all_names = list(in_names) + list(out_names)
    if partition_name is not None:
        all_names.append(partition_name)
    donate = tuple(range(n_params, n_params + len(out_names)))

    def _body(*args):
        operands = list(args)
        if partition_name is not None:
            operands.append(bass2jax.partition_id_tensor())
        outs = bass2jax._bass_exec_p.bind(
            *operands,
            out_avals=tuple(out_avals),
            in_names=tuple(all_names),
            out_names=tuple(out_names),
            lowering_input_output_aliases=(),
            sim_require_finite=True,
            sim_require_nnan=True,
            nc=nc,
        )
        return tuple(outs)

    devices = jax.devices()[:NCORES]
    assert len(devices) == NCORES
    mesh = Mesh(np.asarray(devices), ("core",))
    in_specs = (PartitionSpec("core"),) * (n_params + len(out_names))
    out_specs = (PartitionSpec("core"),) * len(out_names)
    sharded = jax.jit(
        shard_map(
            _body, mesh=mesh, in_specs=in_specs, out_specs=out_specs, check_rep=False
        ),
        donate_argnums=donate,
        keep_unused=True,
    )

    def run(in_maps):
        concat_in = [
            np.concatenate([np.asarray(in_maps[c][nm]) for c in range(NCORES)], axis=0)
            for nm in in_names
        ]
        concat_zeros = [
            np.zeros((NCORES * s[0], *s[1:]), dt) for s, dt in out_shapes
        ]
        out_arrs = sharded(*concat_in, *concat_zeros)
        return [
            {
                nm: np.asarray(out_arrs[i]).reshape(NCORES, *out_shapes[i][0])[c]
                for i, nm in enumerate(out_names)
            }
            for c in range(NCORES)
        ]

    _runner_cache[key] = run
    return run


def _route(x2d: np.ndarray, gate_w: np.ndarray, gate_b: np.ndarray):
    """Top-2 routing on the host (f64 logits for stable ordering)."""
    lg = x2d.astype(np.float64) @ gate_w.astype(np.float64).T
    lg += gate_b.astype(np.float64)
    order = np.argsort(-lg, axis=1, kind="stable")
    ti = order[:, :TOPK]
    tv = np.take_along_axis(lg, ti, axis=1)
    m = tv.max(axis=1, keepdims=True)
    ew = np.exp(tv - m)
    wk = ew / ew.sum(axis=1, keepdims=True)
    return ti, wk


def _tile_kxm(a: np.ndarray, ktiles: int) -> np.ndarray:
    """[Kdim, M] -> [128, ktiles, M] with Kdim = ktiles*128 on partitions."""
    kdim, m = a.shape
    assert kdim == ktiles * P
    return np.ascontiguousarray(a.reshape(ktiles, P, m).transpose(1, 0, 2))


def _tile_w01(w: np.ndarray) -> np.ndarray:
    """[H, D] fp8 weight (pre-quantized) -> [128, KH, KD, 128] h-tile-major."""
    a = _tile_kxm(np.ascontiguousarray(w.T), KD)  # [P, KD, H]
    return np.ascontiguousarray(
        a.reshape(P, KD, KH, P).transpose(0, 2, 1, 3)
    )


def _hi_lo(a: np.ndarray, s: float):
    """fp32 array -> (q8(s*a), q8(s*a - hi)) both e4m3."""
    sa = (a * s).astype(np.float32)
    hi = sa.astype(F8)
    lo = (sa - hi.astype(np.float32)).astype(F8)
    return hi, lo


def _prepare(x, gate_w, gate_b, w0, b0, w1, b1, w2, b2):
    """Host-side routing + per-core input packing. Returns (in_maps, meta)."""
    x = np.asarray(x)
    gate_w = np.asarray(gate_w, dtype=np.float32)
    gate_b = np.asarray(gate_b, dtype=np.float32)
    w0 = np.asarray(w0, dtype=np.float32)
    b0 = np.asarray(b0, dtype=np.float32)
    w1 = np.asarray(w1, dtype=np.float32)
    b1 = np.asarray(b1, dtype=np.float32)
    w2 = np.asarray(w2, dtype=np.float32)
    b2 = np.asarray(b2, dtype=np.float32)

    Bn, Sq, Dv = x.shape
    T = Bn * Sq
    x2d = np.ascontiguousarray(x.reshape(T, Dv)).astype(np.float32, copy=False)

    ti, wk = _route(x2d, gate_w, gate_b)

    idxs, wgts = [], []
    for e in range(E):
        sel = [np.nonzero(ti[:, k] == e)[0] for k in range(TOPK)]
        idxs.append(np.concatenate(sel))
        wgts.append(np.concatenate([wk[s, k] for k, s in enumerate(sel)]))

    maxc = max(len(i) for i in idxs)
    C = max(P, maxc)

    in_maps = []
    for e in range(E):
        xg = np.zeros((C, Dv), dtype=np.float32)
        xg[: len(idxs[e])] = x2d[idxs[e]]
        xh, xl = _hi_lo(xg.T, 1.0)
        w0h, w0l = _hi_lo(w0[e], SW)
        w1h, w1l = _hi_lo(w1[e], SW)
        w2h, w2l = _hi_lo(w2[e], SW)
        in_maps.append(
            {
                "xth": _tile_kxm(np.ascontiguousarray(xh), KD),
                "xtl": _tile_kxm(np.ascontiguousarray(xl), KD),
                "w0h": _tile_w01(w0h),
                "w0l": _tile_w01(w0l),
                "w1h": _tile_w01(w1h),
                "w1l": _tile_w01(w1l),
                "w2h": _tile_kxm(np.ascontiguousarray(w2h.T), KH),
                "w2l": _tile_kxm(np.ascontiguousarray(w2l.T), KH),
                "b0s": np.ascontiguousarray((b0[e] * SW).reshape(KH, P).T),
                "b1t": np.ascontiguousarray(b1[e].reshape(KH, P).T),
            }
        )
    meta = (Bn, Sq, Dv, T, C, idxs, wgts, b2)
    return in_maps, meta


def _combine(results, meta):
    Bn, Sq, Dv, T, C, idxs, wgts, b2 = meta
    out = np.zeros((T, Dv), dtype=np.float32)
    for e in range(E):
        n = len(idxs[e])
        # out_t [128, KD, C] -> [C, D] with d = k*128 + p
        ot = np.asarray(results[e]["out"]).astype(np.float32)
        o = ot.transpose(2, 1, 0).reshape(C, Dv)[:n]
        out[idxs[e]] += wgts[e][:, None].astype(np.float32) * (o + b2[e][None, :])
    return out.reshape(Bn, Sq, Dv)


def kernel(x, gate_w, gate_b, w0, b0, w1, b1, w2, b2):
    in_maps, meta = _prepare(x, gate_w, gate_b, w0, b0, w1, b1, w2, b2)
    C = meta[4]
    run = _get_runner(C)
    try:
        results = run(in_maps)
    except Exception:
        # transient device hiccups happen on the tunneled cores; retry once
        import time as _time

        _time.sleep(2.0)
        try:
            results = run(in_maps)
        except Exception:
            # last resort: rebuild the PJRT client + executable from scratch
            import jax

            _runner_cache.clear()
            try:
                jax.clear_caches()
                jax.extend.backend.clear_backends()
            except Exception:
                pass
            _time.sleep(5.0)
            results = _get_runner(C)(in_maps)
    return _combine(results, meta)


# revision 51
# speedup vs baseline: 1.0008x; 1.0008x over previous
"""MoE (top-2 of 8 experts, SwiGLU FFN) on 8 Trainium2 NeuronCores.

Strategy: expert-parallel. Routing (gate matmul + top-2 + softmax) is done
on the host in numpy; tokens are gathered per expert, padded to a common
capacity C, and each core runs the full SwiGLU FFN for one expert's tokens.
The host scatters the per-core outputs back with the gate weights.

Matmuls use fp8(e4m3) DoubleRow mode (0.5 PE cycles/row, 2 contraction
slices per instruction) with a 3-term hi/lo decomposition so accuracy
stays at bf16 level while the PE runs 1.33x faster than bf16:

    W @ x ~= W_hi@x_hi + W_hi@x_lo + W_lo@x_hi      (all e4m3)

where W_hi = q8(SW*W), W_lo = q8(SW*W - W_hi) (SW=32 keeps the small
weights out of e4m3's subnormal range), x_hi = q8(x), x_lo = q8(x - x_hi).
All three terms share the PSUM scale SW so they accumulate in one group;
the 1/SW descale is folded into the silu activation's input scale. The
activations are split the same way on device (act scale SA=8; the hi part
is captured by an Act-engine Copy with scale SA/SW, the lo part by a DVE
scalar_tensor_tensor; the last h-pair keeps the whole chain on DVE to cut
cross-engine hops off stage 2's critical path), and stage 2's output
descale 1/(SW*SA) is folded into its PSUM->SBUF copy, emitted in bf16.

Device layouts (per core, pre-tiled on host so every DMA is contiguous):
  xth/xtl   [128, KD, C]      f8e4  xT hi/lo tiles
  w0h/w0l   [128, KH, KD,128] f8e4  w0[e].T h-tile-major hi/lo
  w1h/w1l   [128, KH, KD,128] f8e4
  w2h/w2l   [128, KH, D]      f8e4  w2[e].T tiles (h on part, d on free)
  b0s/b1t   [128, KH]         fp32  b0*SW and b1 per-partition biases
  out       [128, KD, C]      bf16  transposed: out[p,k,c]=ffn_out[c,k*128+p]

Stage 1 computes h0^T/h1^T (h on partitions, tokens on free dim) so that
stage 2 can contract over h without any on-device transpose; stage 2 keeps
d on partitions so its moving dim is the (possibly ragged) token chunk.
"""

import os

import numpy as np
import ml_dtypes

# The tunneled trn2 cores occasionally come up wedged from a prior process;
# asking the runtime to reset cores on init recovers them.
os.environ.setdefault("NEURON_RT_RESET_CORES", "1")

E, TOPK, D, H = 8, 2, 1024, 2048
NCORES = 8
P = 128
KD = D // P   # 8 d-tiles
KH = H // P   # 16 h-tiles
BF16 = ml_dtypes.bfloat16
F8 = ml_dtypes.float8_e4m3
SW = 32.0     # weight quantization scale
SA = 8.0      # activation quantization scale

_build_cache: dict = {}
_ACT_SILU = True  # CoreSim lacks Silu; tests may flip this to Tanh
_N_WARM = 4       # PE warm-up matmuls (cover p-state ramp + first DMAs)
_TAIL = 164       # last-chunk width (short kernel drain, still PE-bound)
_LASTP = 24       # final out piece width (bounds the kernel drain chain)


def _chunk_plan(C: int):
    """Token-chunk widths. A full 512 chunk FIRST so the PE consumes h-tiles
    slower than the serialized DMA stream delivers weight pieces; a small
    TAIL chunk last so the kernel drain (last copies + out DMA) is short.
    Middle chunks must stay >= ~308 so the Act/DVE epilogue per h-tile
    doesn't become the pipeline bottleneck."""
    TAIL = _TAIL
    MINMID = 308
    if C <= 512:
        return [C]
    if C <= 512 + TAIL:
        return [512, C - 512]
    rem = C - TAIL
    n512 = rem // 512
    mid = rem - n512 * 512
    mids = []
    if mid and mid < MINMID and n512 >= 1:
        n512 -= 1
        a = mid + 512
        mids = [a - a // 2, a // 2] if a > 512 else [a]
    elif mid:
        mids = [mid]
    return [512] * n512 + mids + [TAIL]


def _build_bass(C: int, repeat: int = 1):
    """Build the single-core SPMD Bass program for capacity C."""
    import concourse.bacc as bacc
    import concourse.mybir as mybir
    from concourse import tile

    fp32 = mybir.dt.float32
    bf16 = mybir.dt.bfloat16
    f8e4 = mybir.dt.float8e4
    AF = mybir.ActivationFunctionType
    ALU = mybir.AluOpType
    DR = mybir.MatmulPerfMode.DoubleRow

    # token chunks (free dim of the matmuls); any width <= 512. A smaller
    # first chunk lets the PE start while the bulk DMA is still in flight.
    chunks = _chunk_plan(C)

    nc = bacc.Bacc("TRN2", target_bir_lowering=False)
    xth_d = nc.dram_tensor("xth", [P, KD, C], f8e4, kind="ExternalInput")
    xtl_d = nc.dram_tensor("xtl", [P, KD, C], f8e4, kind="ExternalInput")
    # w0/w1 are h-tile-major so any h-tile piece is a fully contiguous DMA
    w0h_d = nc.dram_tensor("w0h", [P, KH, KD, P], f8e4, kind="ExternalInput")
    w0l_d = nc.dram_tensor("w0l", [P, KH, KD, P], f8e4, kind="ExternalInput")
    w1h_d = nc.dram_tensor("w1h", [P, KH, KD, P], f8e4, kind="ExternalInput")
    w1l_d = nc.dram_tensor("w1l", [P, KH, KD, P], f8e4, kind="ExternalInput")
    w2h_d = nc.dram_tensor("w2h", [P, KH, D], f8e4, kind="ExternalInput")
    w2l_d = nc.dram_tensor("w2l", [P, KH, D], f8e4, kind="ExternalInput")
    b0s_d = nc.dram_tensor("b0s", [P, KH], fp32, kind="ExternalInput")
    b1t_d = nc.dram_tensor("b1t", [P, KH], fp32, kind="ExternalInput")
    # out is produced transposed: out_t[p, k, c] = ffn_out[c, k*128+p]
    out_d = nc.dram_tensor("out", [P, KD, C], bf16, kind="ExternalOutput")

    with tile.TileContext(nc) as tc:
        with (
            tc.tile_pool(name="wconst", bufs=1) as wpool,
            tc.tile_pool(name="xtp", bufs=1) as xpool,
            tc.tile_pool(name="act", bufs=3) as apool,
            tc.tile_pool(name="sil", bufs=4) as spool,
            tc.tile_pool(name="af32", bufs=4) as fpool,
            tc.tile_pool(name="osb", bufs=6) as opool,
            tc.tile_pool(name="ps0", bufs=2, space="PSUM") as pp0,
            tc.tile_pool(name="ps1", bufs=2, space="PSUM") as pp1,
            tc.tile_pool(name="pso", bufs=3, space="PSUM") as ppo,
            tc.tile_pool(name="warm", bufs=1, space="PSUM") as wppool,
        ):
            w0h_sb = wpool.tile([P, KH, KD, P], f8e4, tag="w0h")
            w0l_sb = wpool.tile([P, KH, KD, P], f8e4, tag="w0l")
            w1h_sb = wpool.tile([P, KH, KD, P], f8e4, tag="w1h")
            w1l_sb = wpool.tile([P, KH, KD, P], f8e4, tag="w1l")
            w2h_sb = wpool.tile([P, KH, D], f8e4, tag="w2h")
            w2l_sb = wpool.tile([P, KH, D], f8e4, tag="w2l")
            b0s_sb = wpool.tile([P, KH], fp32, tag="b0s")
            b1t_sb = wpool.tile([P, KH], fp32, tag="b1t")
            # Warm the PE (HAM clock gate / p-state ramp) with dummy matmuls
            # on a zeroed tile while the first weight/token DMAs are in
            # flight — the ramp to full clock happens before real work.
            z_sb = wpool.tile([P, P], bf16, tag="warmz")
            nc.scalar.memzero(z_sb[:])
            zp = wppool.tile([P, P], fp32, tag="warmp")
            n_warm = _N_WARM if C >= 768 else 16
            for _ in range(n_warm):
                nc.tensor.matmul(zp[:], z_sb[:], z_sb[:], start=True, stop=True)

            # xt streams per chunk through the pool (one buf per chunk so xt
            # DMAs never wait on tile reuse and can't block the DGE queue).
            xt_tiles = []
            # Load order matters: transfers drain serially in issue order, so
            # deliver exactly in first-use order: (w1h,xh,w1l,w0h,w0l,xl) for
            # h-piece 0, then the remaining h-pieces, then w2 d-halves, then
            # the later token chunks.
            hpieces = [(0, 2), (2, 2), (4, 2), (6, 2), (8, 4), (12, 4)]
            j0_, jw_ = hpieces[0]
            js0 = slice(j0_, j0_ + jw_)
            nc.sync.dma_start(w1h_sb[:, js0], w1h_d[:, js0])
            xth0 = xpool.tile([P, KD, chunks[0]], f8e4, tag="xth0")
            xtl0 = xpool.tile([P, KD, chunks[0]], f8e4, tag="xtl0")
            hd = KD // 2
            nc.sync.dma_start(xth0[:, 0:hd], xth_d[:, 0:hd, 0:chunks[0]])
            nc.sync.dma_start(xth0[:, hd:], xth_d[:, hd:, 0:chunks[0]])
            nc.sync.dma_start(w1l_sb[:, js0], w1l_d[:, js0])
            nc.sync.dma_start(xtl0[:, 0:hd], xtl_d[:, 0:hd, 0:chunks[0]])
            nc.sync.dma_start(xtl0[:, hd:], xtl_d[:, hd:, 0:chunks[0]])
            nc.sync.dma_start(b1t_sb[:], b1t_d[:])
            nc.sync.dma_start(w0h_sb[:, js0], w0h_d[:, js0])
            nc.sync.dma_start(w0l_sb[:, js0], w0l_d[:, js0])
            nc.sync.dma_start(b0s_sb[:], b0s_d[:])
            xt_tiles.append((xth0, xtl0))
            for j0_, jw_ in hpieces[1:]:
                js_ = slice(j0_, j0_ + jw_)
                nc.sync.dma_start(w1h_sb[:, js_], w1h_d[:, js_])
                nc.sync.dma_start(w1l_sb[:, js_], w1l_d[:, js_])
                nc.sync.dma_start(w0h_sb[:, js_], w0h_d[:, js_])
                nc.sync.dma_start(w0l_sb[:, js_], w0l_d[:, js_])
            # w2 streamed in d-halves (512B lines keep full DMA throughput),
            # ordered hi.a, lo.a, hi.b, lo.b to match stage-2 d-tile order.
            nc.sync.dma_start(w2h_sb[:, :, 0:512], w2h_d[:, :, 0:512])
            nc.sync.dma_start(w2l_sb[:, :, 0:512], w2l_d[:, :, 0:512])
            nc.sync.dma_start(w2h_sb[:, :, 512:D], w2h_d[:, :, 512:D])
            nc.sync.dma_start(w2l_sb[:, :, 512:D], w2l_d[:, :, 512:D])
            cpos = chunks[0]
            for ci_, tcw_ in enumerate(chunks[1:]):
                xth_ch = xpool.tile([P, KD, tcw_], f8e4, tag=f"xth{ci_ + 1}")
                xtl_ch = xpool.tile([P, KD, tcw_], f8e4, tag=f"xtl{ci_ + 1}")
                nc.sync.dma_start(xth_ch[:], xth_d[:, :, cpos:cpos + tcw_])
                nc.sync.dma_start(xtl_ch[:], xtl_d[:, :, cpos:cpos + tcw_])
                xt_tiles.append((xth_ch, xtl_ch))
                cpos += tcw_

            def _s1_part(ps, w_sb, ht, x_sb, start, stop):
                """4 DoubleRow matmuls: one term of a stage-1 group."""
                for j in range(KD // 2):
                    nc.tensor.matmul(
                        ps[:],
                        w_sb[:, ht, 2 * j:2 * j + 2, :],
                        x_sb[:, 2 * j:2 * j + 2, :],
                        start=start and j == 0,
                        stop=stop and j == KD // 2 - 1,
                        perf_mode=DR,
                    )

            def _s1_group(ps, wh_sb, wl_sb, ht, xh, xl, skip_first=False):
                """12 DoubleRow matmuls: Wh@xh + Wl@xh + Wh@xl (scale SW).
                With skip_first the Wh@xh term was already emitted early."""
                if not skip_first:
                    _s1_part(ps, wh_sb, ht, xh, True, False)
                _s1_part(ps, wl_sb, ht, xh, False, False)
                _s1_part(ps, wh_sb, ht, xl, False, True)

            def _body():
                c0 = 0
                for ci, tcw in enumerate(chunks):
                    xh_sb, xl_sb = xt_tiles[ci]
                    # per-h-pair act tiles: stage 2's pair-j matmul depends
                    # only on pair j's epilogue, so it can start while the
                    # last h-tiles' act split is still in flight.
                    ah_sb = [apool.tile([P, 2, tcw], f8e4, tag=f"ah{j}",
                                        name=f"ah{j}_{ci}")
                             for j in range(KH // 2)]
                    al_sb = [apool.tile([P, 2, tcw], f8e4, tag=f"al{j}",
                                        name=f"al{j}_{ci}")
                             for j in range(KH // 2)]
                    # For the very first chunk, emit the Wh@xh term of the
                    # first two h-tiles' groups up front: it only needs the
                    # first weight piece + x_hi, so the PE has useful work
                    # while w_lo / x_lo are still streaming in.
                    pre = {}
                    if ci == 0:
                        for ht in (0, 1):
                            ps1 = pp1.tile([P, tcw], fp32, tag="ps1")
                            _s1_part(ps1, w1h_sb, ht, xh_sb, True, False)
                            ps0 = pp0.tile([P, tcw], fp32, tag="ps0")
                            _s1_part(ps0, w0h_sb, ht, xh_sb, True, False)
                            pre[ht] = (ps1, ps0)
                    for ht in range(KH):
                        if ht in pre:
                            ps1, ps0 = pre[ht]
                            _s1_group(ps1, w1h_sb, w1l_sb, ht, xh_sb, xl_sb,
                                      skip_first=True)
                            _s1_group(ps0, w0h_sb, w0l_sb, ht, xh_sb, xl_sb,
                                      skip_first=True)
                        else:
                            ps1 = pp1.tile([P, tcw], fp32, tag="ps1")
                            _s1_group(ps1, w1h_sb, w1l_sb, ht, xh_sb, xl_sb)
                            ps0 = pp0.tile([P, tcw], fp32, tag="ps0")
                            _s1_group(ps0, w0h_sb, w0l_sb, ht, xh_sb, xl_sb)
                        sil = spool.tile([P, tcw], fp32, tag="sil")
                        af = AF.Silu if _ACT_SILU else AF.Tanh
                        # sil = silu(h1 + b1) with h1 = ps1/SW
                        nc.scalar.activation(
                            sil[:], ps1[:], af,
                            bias=b1t_sb[:, ht:ht + 1], scale=1.0 / SW,
                        )
                        # af32 = (ps0 + SW*b0) * sil = SW * act
                        last_pair = ht >= KH - 2
                        af32 = fpool.tile([P, tcw], fp32, tag="af32")
                        nc.vector.scalar_tensor_tensor(
                            af32[:], ps0[:], b0s_sb[:, ht:ht + 1], sil[:],
                            ALU.add, ALU.mult,
                        )
                        # act_hi = q8(SA * act), scale SA/SW
                        ah_t = ah_sb[ht // 2][:, ht % 2, :]
                        if last_pair:
                            # keep the whole chain on DVE: no cross-engine
                            # sem hops on the path stage 2's last pair waits on
                            nc.vector.tensor_scalar(
                                ah_t, af32[:], SA / SW, None, ALU.mult,
                            )
                        else:
                            nc.scalar.activation(
                                ah_t, af32[:], AF.Copy, scale=SA / SW,
                            )
                        # act_lo = q8(SA*act - act_hi)
                        nc.vector.scalar_tensor_tensor(
                            al_sb[ht // 2][:, ht % 2, :], af32[:], SA / SW,
                            ah_t, ALU.mult, ALU.subtract,
                        )
                    # stage 2 transposed: out_t[d-tile] = w2T_tile.T @ act
                    last_chunk = c0 + tcw == C
                    for dk in range(KD):
                        # split the very last group into token-halves so the
                        # first half's copy+DMA overlaps the second half's
                        # matmuls (shorter kernel tail)
                        if last_chunk and dk == KD - 1 and tcw >= 128:
                            halves = [(0, tcw - _LASTP), (tcw - _LASTP, _LASTP)]
                        else:
                            halves = [(0, tcw)]
                        ds = slice(dk * P, (dk + 1) * P)
                        for f0, fw in halves:
                            pso = ppo.tile([P, fw], fp32, tag="pso")
                            terms = [(w2h_sb, ah_sb), (w2l_sb, ah_sb),
                                     (w2h_sb, al_sb)]
                            # j-major so pair j's act tiles are needed as
                            # late as possible: the last h-pair's epilogue
                            # (still in flight right after stage 1) is only
                            # read by matmuls 21..23 of the group.
                            n = len(terms) * (KH // 2)
                            i = 0
                            for j in range(KH // 2):
                                for w_sb, a_sb in terms:
                                    nc.tensor.matmul(
                                        pso[:],
                                        w_sb[:, 2 * j:2 * j + 2, ds],
                                        a_sb[j][:, :, f0:f0 + fw],
                                        start=(i == 0),
                                        stop=(i == n - 1),
                                        perf_mode=DR,
                                    )
                                    i += 1
                            o_sb = opool.tile([P, fw], bf16, tag="osb")
                            # descale 1/(SW*SA) folded into the PSUM->SBUF copy
                            nc.scalar.activation(
                                o_sb[:], pso[:], AF.Copy, scale=1.0 / (SW * SA),
                            )
                            nc.sync.dma_start(
                                out_d[:, dk, c0 + f0:c0 + f0 + fw], o_sb[:]
                            )
                    c0 += tcw

            if repeat == 1:
                _body()
            else:
                # hardware loop: constant program size for any repeat count
                # (used only for wall-clock benchmarking of the body)
                with tc.For_i(0, repeat, 1):
                    _body()
    nc.compile()
    return nc


def _get_bass(C: int, repeat: int = 1):
    key = (C, repeat)
    if key not in _build_cache:
        _build_cache[key] = _build_bass(C, repeat)
    return _build_cache[key]


_runner_cache: dict = {}


def _get_runner(C: int, repeat: int = 1):
    """Compile the SPMD program once and return a reusable launcher.

    Mirrors concourse.bass2jax.run_bass_via_pjrt but memoizes the jitted
    executable so repeated kernel() calls don't recompile the NEFF.
    """
    key = (C, repeat)
    if key in _runner_cache:
        return _runner_cache[key]

    import jax
    from jax.experimental.shard_map import shard_map
    from jax.sharding import Mesh, PartitionSpec
    import concourse.mybir as mybir
    from concourse import bass2jax

    nc = _get_bass(C, repeat)
    bass2jax.install_neuronx_cc_hook()
    partition_name = nc.partition_id_tensor.name if nc.partition_id_tensor else None

    in_names: list = []
    out_names: list = []
    out_avals: list = []
    out_shapes: list = []
    for alloc in nc.m.functions[0].allocations:
        if not isinstance(alloc, mybir.MemoryLocationSet):
            continue
        name = alloc.memorylocations[0].name
        if alloc.kind == "ExternalInput":
            if name != partition_name:
                in_names.append(name)
        elif alloc.kind == "ExternalOutput":
            shape = tuple(alloc.tensor_shape)
            dtype = mybir.dt.np(alloc.dtype)
            out_names.append(name)
            out_avals.append(jax.core.ShapedArray(shape, dtype))
            out_shapes.append((shape, dtype))
    n_params = len(in_names)
    all_names = list(in_names) + list(out_names)
    if partition_name is not None:
        all_names.append(partition_name)
    donate = tuple(range(n_params, n_params + len(out_names)))

    def _body(*args):
        operands = list(args)
        if partition_name is not None:
            operands.append(bass2jax.partition_id_tensor())
        outs = bass2jax._bass_exec_p.bind(
            *operands,
            out_avals=tuple(out_avals),
            in_names=tuple(all_names),
            out_names=tuple(out_names),
            lowering_input_output_aliases=(),
            sim_require_finite=True,
            sim_require_nnan=True,
            nc=nc,
        )
        return tuple(outs)

    devices = jax.devices()[:NCORES]
    assert len(devices) == NCORES
    mesh = Mesh(np.asarray(devices), ("core",))
    in_specs = (PartitionSpec("core"),) * (n_params + len(out_names))
    out_specs = (PartitionSpec("core"),) * len(out_names)
    sharded = jax.jit(
        shard_map(
            _body, mesh=mesh, in_specs=in_specs, out_specs=out_specs, check_rep=False
        ),
        donate_argnums=donate,
        keep_unused=True,
    )

    def run(in_maps):
        concat_in = [
            np.concatenate([np.asarray(in_maps[c][nm]) for c in range(NCORES)], axis=0)
            for nm in in_names
        ]
        concat_zeros = [
            np.zeros((NCORES * s[0], *s[1:]), dt) for s, dt in out_shapes
        ]
        out_arrs = sharded(*concat_in, *concat_zeros)
        return [
            {
                nm: np.asarray(out_arrs[i]).reshape(NCORES, *out_shapes[i][0])[c]
                for i, nm in enumerate(out_names)
            }
            for c in range(NCORES)
        ]

    _runner_cache[key] = run
    return run


def _route(x2d: np.ndarray, gate_w: np.ndarray, gate_b: np.ndarray):
    """Top-2 routing on the host (f64 logits for stable ordering)."""
    lg = x2d.astype(np.float64) @ gate_w.astype(np.float64).T
    lg += gate_b.astype(np.float64)
    order = np.argsort(-lg, axis=1, kind="stable")
    ti = order[:, :TOPK]
    tv = np.take_along_axis(lg, ti, axis=1)
    m = tv.max(axis=1, keepdims=True)
    ew = np.exp(tv - m)
    wk = ew / ew.sum(axis=1, keepdims=True)
    return ti, wk


def _tile_kxm(a: np.ndarray, ktiles: int) -> np.ndarray:
    """[Kdim, M] -> [128, ktiles, M] with Kdim = ktiles*128 on partitions."""
    kdim, m = a.shape
    assert kdim == ktiles * P
    return np.ascontiguousarray(a.reshape(ktiles, P, m).transpose(1, 0, 2))


def _tile_w01(w: np.ndarray) -> np.ndarray:
    """[H, D] fp8 weight (pre-quantized) -> [128, KH, KD, 128] h-tile-major."""
    a = _tile_kxm(np.ascontiguousarray(w.T), KD)  # [P, KD, H]
    return np.ascontiguousarray(
        a.reshape(P, KD, KH, P).transpose(0, 2, 1, 3)
    )


def _hi_lo(a: np.ndarray, s: float):
    """fp32 array -> (q8(s*a), q8(s*a - hi)) both e4m3."""
    sa = (a * s).astype(np.float32)
    hi = sa.astype(F8)
    lo = (sa - hi.astype(np.float32)).astype(F8)
    return hi, lo


def _prepare(x, gate_w, gate_b, w0, b0, w1, b1, w2, b2):
    """Host-side routing + per-core input packing. Returns (in_maps, meta)."""
    x = np.asarray(x)
    gate_w = np.asarray(gate_w, dtype=np.float32)
    gate_b = np.asarray(gate_b, dtype=np.float32)
    w0 = np.asarray(w0, dtype=np.float32)
    b0 = np.asarray(b0, dtype=np.float32)
    w1 = np.asarray(w1, dtype=np.float32)
    b1 = np.asarray(b1, dtype=np.float32)
    w2 = np.asarray(w2, dtype=np.float32)
    b2 = np.asarray(b2, dtype=np.float32)

    Bn, Sq, Dv = x.shape
    T = Bn * Sq
    x2d = np.ascontiguousarray(x.reshape(T, Dv)).astype(np.float32, copy=False)

    ti, wk = _route(x2d, gate_w, gate_b)

    idxs, wgts = [], []
    for e in range(E):
        sel = [np.nonzero(ti[:, k] == e)[0] for k in range(TOPK)]
        idxs.append(np.concatenate(sel))
        wgts.append(np.concatenate([wk[s, k] for k, s in enumerate(sel)]))

    maxc = max(len(i) for i in idxs)
    C = max(P, maxc)

    in_maps = []
    for e in range(E):
        xg = np.zeros((C, Dv), dtype=np.float32)
        xg[: len(idxs[e])] = x2d[idxs[e]]
        xh, xl = _hi_lo(xg.T, 1.0)
        w0h, w0l = _hi_lo(w0[e], SW)
        w1h, w1l = _hi_lo(w1[e], SW)
        w2h, w2l = _hi_lo(w2[e], SW)
        in_maps.append(
            {
                "xth": _tile_kxm(np.ascontiguousarray(xh), KD),
                "xtl": _tile_kxm(np.ascontiguousarray(xl), KD),
                "w0h": _tile_w01(w0h),
                "w0l": _tile_w01(w0l),
                "w1h": _tile_w01(w1h),
                "w1l": _tile_w01(w1l),
                "w2h": _tile_kxm(np.ascontiguousarray(w2h.T), KH),
                "w2l": _tile_kxm(np.ascontiguousarray(w2l.T), KH),
                "b0s": np.ascontiguousarray((b0[e] * SW).reshape(KH, P).T),
                "b1t": np.ascontiguousarray(b1[e].reshape(KH, P).T),
            }
        )
    meta = (Bn, Sq, Dv, T, C, idxs, wgts, b2)
    return in_maps, meta


def _combine(results, meta):
    Bn, Sq, Dv, T, C, idxs, wgts, b2 = meta
    out = np.zeros((T, Dv), dtype=np.float32)
    for e in range(E):
        n = len(idxs[e])
        # out_t [128, KD, C] -> [C, D] with d = k*128 + p
        ot = np.asarray(results[e]["out"]).astype(np.float32)
        o = ot.transpose(2, 1, 0).reshape(C, Dv)[:n]
        out[idxs[e]] += wgts[e][:, None].astype(np.float32) * (o + b2[e][None, :])
    return out.reshape(Bn, Sq, Dv)


def kernel(x, gate_w, gate_b, w0, b0, w1, b1, w2, b2):
    in_maps, meta = _prepare(x, gate_w, gate_b, w0, b0, w1, b1, w2, b2)
    C = meta[4]
    results = None
    last_err = None
    # The tunneled cores occasionally wedge mid-run; plain retries, then
    # retries that rebuild the PJRT client + executable from scratch.
    for attempt, (delay, rebuild) in enumerate(
        [(0.0, False), (2.0, False), (10.0, True), (20.0, True)]
    ):
        import time as _time

        if delay:
            _time.sleep(delay)
        if rebuild:
            import jax

            _runner_cache.clear()
            try:
                jax.clear_caches()
                jax.extend.backend.clear_backends()
            except Exception:
                pass
        try:
            results = _get_runner(C)(in_maps)
            break
        except Exception as e:  # noqa: BLE001
            last_err = e
    if results is None:
        raise last_err
    return _combine(results, meta)


# revision 53
# speedup vs baseline: 1.0227x; 1.0219x over previous
"""MoE (top-2 of 8 experts, SwiGLU FFN) on 8 Trainium2 NeuronCores.

Strategy: expert-parallel. Routing (gate matmul + top-2 + softmax) is done
on the host in numpy; tokens are gathered per expert, padded to a common
capacity C, and each core runs the full SwiGLU FFN for one expert's tokens.
The host scatters the per-core outputs back with the gate weights.

Matmuls use fp8(e4m3) DoubleRow mode (0.5 PE cycles/row, 2 contraction
slices per instruction) with a 3-term hi/lo decomposition so accuracy
stays at bf16 level while the PE runs 1.33x faster than bf16:

    W @ x ~= W_hi@x_hi + W_hi@x_lo + W_lo@x_hi      (all e4m3)

where W_hi = q8(SW*W), W_lo = q8(SW*W - W_hi) (SW=32 keeps the small
weights out of e4m3's subnormal range), x_hi = q8(x), x_lo = q8(x - x_hi).
All three terms share the PSUM scale SW so they accumulate in one group;
the 1/SW descale is folded into the silu activation's input scale. The
activations are split the same way on device (act scale SA=8; the hi part
is captured by an Act-engine Copy with scale SA/SW, the lo part by a DVE
scalar_tensor_tensor; the last h-pair keeps the whole chain on DVE to cut
cross-engine hops off stage 2's critical path), and stage 2's output
descale 1/(SW*SA) is folded into its PSUM->SBUF copy, emitted in bf16.

Device layouts (per core, pre-tiled on host so every DMA is contiguous):
  xth/xtl   [128, KD, C]      f8e4  xT hi/lo tiles
  w0h/w0l   [128, KH, KD,128] f8e4  w0[e].T h-tile-major hi/lo
  w1h/w1l   [128, KH, KD,128] f8e4
  w2h/w2l   [128, KH, D]      f8e4  w2[e].T tiles (h on part, d on free)
  b0s/b1t   [128, KH]         fp32  b0*SW and b1 per-partition biases
  out       [128, KD, C]      bf16  transposed: out[p,k,c]=ffn_out[c,k*128+p]

Stage 1 computes h0^T/h1^T (h on partitions, tokens on free dim) so that
stage 2 can contract over h without any on-device transpose; stage 2 keeps
d on partitions so its moving dim is the (possibly ragged) token chunk.
"""

import os

import numpy as np
import ml_dtypes

# The tunneled trn2 cores occasionally come up wedged from a prior process;
# asking the runtime to reset cores on init recovers them.
os.environ.setdefault("NEURON_RT_RESET_CORES", "1")

E, TOPK, D, H = 8, 2, 1024, 2048
NCORES = 8
P = 128
KD = D // P   # 8 d-tiles
KH = H // P   # 16 h-tiles
BF16 = ml_dtypes.bfloat16
F8 = ml_dtypes.float8_e4m3
SW = 32.0     # weight quantization scale
SA = 8.0      # activation quantization scale

_build_cache: dict = {}
_ACT_SILU = True  # CoreSim lacks Silu; tests may flip this to Tanh
_N_WARM = 4       # PE warm-up matmuls (cover p-state ramp + first DMAs)
_TAIL = 164       # last-chunk width (short kernel drain, still PE-bound)
_LASTP = 24       # final out piece width (bounds the kernel drain chain)


def _chunk_plan(C: int):
    """Token-chunk widths. A full 512 chunk FIRST so the PE consumes h-tiles
    slower than the serialized DMA stream delivers weight pieces; a small
    TAIL chunk last so the kernel drain (last copies + out DMA) is short.
    Middle chunks must stay >= ~308 so the Act/DVE epilogue per h-tile
    doesn't become the pipeline bottleneck."""
    TAIL = _TAIL
    MINMID = 308
    if C <= 512:
        return [C]
    if C <= 512 + TAIL:
        return [512, C - 512]
    rem = C - TAIL
    n512 = rem // 512
    mid = rem - n512 * 512
    mids = []
    if mid and mid < MINMID and n512 >= 1:
        n512 -= 1
        a = mid + 512
        mids = [a - a // 2, a // 2] if a > 512 else [a]
    elif mid:
        mids = [mid]
    return [512] * n512 + mids + [TAIL]


def _build_bass(C: int, repeat: int = 1):
    """Build the single-core SPMD Bass program for capacity C."""
    import concourse.bacc as bacc
    import concourse.mybir as mybir
    from concourse import tile

    fp32 = mybir.dt.float32
    bf16 = mybir.dt.bfloat16
    f8e4 = mybir.dt.float8e4
    AF = mybir.ActivationFunctionType
    ALU = mybir.AluOpType
    DR = mybir.MatmulPerfMode.DoubleRow

    # token chunks (free dim of the matmuls); any width <= 512. A smaller
    # first chunk lets the PE start while the bulk DMA is still in flight.
    chunks = _chunk_plan(C)

    nc = bacc.Bacc("TRN2", target_bir_lowering=False)
    xth_d = nc.dram_tensor("xth", [P, KD, C], f8e4, kind="ExternalInput")
    xtl_d = nc.dram_tensor("xtl", [P, KD, C], f8e4, kind="ExternalInput")
    # w0/w1 are h-tile-major so any h-tile piece is a fully contiguous DMA
    w0h_d = nc.dram_tensor("w0h", [P, KH, KD, P], f8e4, kind="ExternalInput")
    w0l_d = nc.dram_tensor("w0l", [P, KH, KD, P], f8e4, kind="ExternalInput")
    w1h_d = nc.dram_tensor("w1h", [P, KH, KD, P], f8e4, kind="ExternalInput")
    w1l_d = nc.dram_tensor("w1l", [P, KH, KD, P], f8e4, kind="ExternalInput")
    w2h_d = nc.dram_tensor("w2h", [P, KH, D], f8e4, kind="ExternalInput")
    w2l_d = nc.dram_tensor("w2l", [P, KH, D], f8e4, kind="ExternalInput")
    b0s_d = nc.dram_tensor("b0s", [P, KH], fp32, kind="ExternalInput")
    b1t_d = nc.dram_tensor("b1t", [P, KH], fp32, kind="ExternalInput")
    # out is produced transposed: out_t[p, k, c] = ffn_out[c, k*128+p]
    out_d = nc.dram_tensor("out", [P, KD, C], bf16, kind="ExternalOutput")

    with tile.TileContext(nc) as tc:
        with (
            tc.tile_pool(name="wconst", bufs=1) as wpool,
            tc.tile_pool(name="xtp", bufs=1) as xpool,
            tc.tile_pool(name="act", bufs=3) as apool,
            tc.tile_pool(name="sil", bufs=4) as spool,
            tc.tile_pool(name="af32", bufs=4) as fpool,
            tc.tile_pool(name="osb", bufs=6) as opool,
            tc.tile_pool(name="ps0", bufs=2, space="PSUM") as pp0,
            tc.tile_pool(name="ps1", bufs=2, space="PSUM") as pp1,
            tc.tile_pool(name="pso", bufs=3, space="PSUM") as ppo,
            tc.tile_pool(name="warm", bufs=1, space="PSUM") as wppool,
        ):
            w0h_sb = wpool.tile([P, KH, KD, P], f8e4, tag="w0h")
            w0l_sb = wpool.tile([P, KH, KD, P], f8e4, tag="w0l")
            w1h_sb = wpool.tile([P, KH, KD, P], f8e4, tag="w1h")
            w1l_sb = wpool.tile([P, KH, KD, P], f8e4, tag="w1l")
            w2h_sb = wpool.tile([P, KH, D], f8e4, tag="w2h")
            w2l_sb = wpool.tile([P, KH, D], f8e4, tag="w2l")
            b0s_sb = wpool.tile([P, KH], fp32, tag="b0s")
            b1t_sb = wpool.tile([P, KH], fp32, tag="b1t")
            # Warm the PE (HAM clock gate / p-state ramp) with dummy matmuls
            # on a zeroed tile while the first weight/token DMAs are in
            # flight — the ramp to full clock happens before real work.
            z_sb = wpool.tile([P, P], bf16, tag="warmz")
            nc.scalar.memzero(z_sb[:])
            zp = wppool.tile([P, P], fp32, tag="warmp")
            n_warm = _N_WARM if C >= 768 else 16
            for _ in range(n_warm):
                nc.tensor.matmul(zp[:], z_sb[:], z_sb[:], start=True, stop=True)

            # xt streams per chunk through the pool (one buf per chunk so xt
            # DMAs never wait on tile reuse and can't block the DGE queue).
            xt_tiles = []
            # Load order matters: transfers drain serially in issue order, so
            # deliver exactly in first-use order: (w1h,xh,w1l,w0h,w0l,xl) for
            # h-piece 0, then the remaining h-pieces, then w2 d-halves, then
            # the later token chunks.
            hpieces = [(0, 2), (2, 2), (4, 2), (6, 2), (8, 4), (12, 4)]
            j0_, jw_ = hpieces[0]
            js0 = slice(j0_, j0_ + jw_)
            nc.sync.dma_start(w1h_sb[:, js0], w1h_d[:, js0])
            xth0 = xpool.tile([P, KD, chunks[0]], f8e4, tag="xth0")
            xtl0 = xpool.tile([P, KD, chunks[0]], f8e4, tag="xtl0")
            hd = KD // 2
            nc.sync.dma_start(xth0[:, 0:hd], xth_d[:, 0:hd, 0:chunks[0]])
            nc.sync.dma_start(xth0[:, hd:], xth_d[:, hd:, 0:chunks[0]])
            nc.sync.dma_start(w1l_sb[:, js0], w1l_d[:, js0])
            nc.sync.dma_start(xtl0[:, 0:hd], xtl_d[:, 0:hd, 0:chunks[0]])
            nc.sync.dma_start(xtl0[:, hd:], xtl_d[:, hd:, 0:chunks[0]])
            nc.sync.dma_start(b1t_sb[:], b1t_d[:])
            nc.sync.dma_start(w0h_sb[:, js0], w0h_d[:, js0])
            nc.sync.dma_start(w0l_sb[:, js0], w0l_d[:, js0])
            nc.sync.dma_start(b0s_sb[:], b0s_d[:])
            xt_tiles.append((xth0, xtl0))
            for j0_, jw_ in hpieces[1:]:
                js_ = slice(j0_, j0_ + jw_)
                nc.sync.dma_start(w1h_sb[:, js_], w1h_d[:, js_])
                nc.sync.dma_start(w1l_sb[:, js_], w1l_d[:, js_])
                nc.sync.dma_start(w0h_sb[:, js_], w0h_d[:, js_])
                nc.sync.dma_start(w0l_sb[:, js_], w0l_d[:, js_])
            # w2 streamed in d-halves (512B lines keep full DMA throughput),
            # ordered hi.a, lo.a, hi.b, lo.b to match stage-2 d-tile order.
            nc.sync.dma_start(w2h_sb[:, :, 0:512], w2h_d[:, :, 0:512])
            nc.sync.dma_start(w2l_sb[:, :, 0:512], w2l_d[:, :, 0:512])
            nc.sync.dma_start(w2h_sb[:, :, 512:D], w2h_d[:, :, 512:D])
            nc.sync.dma_start(w2l_sb[:, :, 512:D], w2l_d[:, :, 512:D])
            cpos = chunks[0]
            for ci_, tcw_ in enumerate(chunks[1:]):
                xth_ch = xpool.tile([P, KD, tcw_], f8e4, tag=f"xth{ci_ + 1}")
                xtl_ch = xpool.tile([P, KD, tcw_], f8e4, tag=f"xtl{ci_ + 1}")
                nc.sync.dma_start(xth_ch[:], xth_d[:, :, cpos:cpos + tcw_])
                nc.sync.dma_start(xtl_ch[:], xtl_d[:, :, cpos:cpos + tcw_])
                xt_tiles.append((xth_ch, xtl_ch))
                cpos += tcw_

            def _s1_part(ps, w_sb, ht, x_sb, start, stop, npairs=KD // 2):
                """DoubleRow matmuls over d-pairs: one term of a s1 group."""
                for j in range(npairs):
                    nc.tensor.matmul(
                        ps[:],
                        w_sb[:, ht, 2 * j:2 * j + 2, :],
                        x_sb[:, 2 * j:2 * j + 2, :],
                        start=start and j == 0,
                        stop=stop and j == npairs - 1,
                        perf_mode=DR,
                    )

            def _s1_group(ps, wh_sb, wl_sb, ht, xh, xl, skip_first=False,
                          xl_pairs=KD // 2):
                """Stage-1 group: Wh@xh + Wl@xh + Wh@xl (scale SW).
                With skip_first the Wh@xh term was already emitted early.
                xl_pairs < KD//2 truncates the x_lo correction term — used
                on the silu branch only, where silu damps the (tiny) extra
                quantization error; measured end-to-end 1.18e-2 vs the 2e-2
                gate."""
                if not skip_first:
                    _s1_part(ps, wh_sb, ht, xh, True, False)
                _s1_part(ps, wl_sb, ht, xh, False, False)
                _s1_part(ps, wh_sb, ht, xl, False, True, npairs=xl_pairs)

            def _body():
                c0 = 0
                for ci, tcw in enumerate(chunks):
                    xh_sb, xl_sb = xt_tiles[ci]
                    # per-h-pair act tiles: stage 2's pair-j matmul depends
                    # only on pair j's epilogue, so it can start while the
                    # last h-tiles' act split is still in flight.
                    ah_sb = [apool.tile([P, 2, tcw], f8e4, tag=f"ah{j}",
                                        name=f"ah{j}_{ci}")
                             for j in range(KH // 2)]
                    al_sb = [apool.tile([P, 2, tcw], f8e4, tag=f"al{j}",
                                        name=f"al{j}_{ci}")
                             for j in range(KH // 2)]
                    # For the very first chunk, emit the Wh@xh term of the
                    # first two h-tiles' groups up front: it only needs the
                    # first weight piece + x_hi, so the PE has useful work
                    # while w_lo / x_lo are still streaming in.
                    pre = {}
                    if ci == 0:
                        for ht in (0, 1):
                            ps1 = pp1.tile([P, tcw], fp32, tag="ps1")
                            _s1_part(ps1, w1h_sb, ht, xh_sb, True, False)
                            ps0 = pp0.tile([P, tcw], fp32, tag="ps0")
                            _s1_part(ps0, w0h_sb, ht, xh_sb, True, False)
                            pre[ht] = (ps1, ps0)
                    for ht in range(KH):
                        if ht in pre:
                            ps1, ps0 = pre[ht]
                            _s1_group(ps1, w1h_sb, w1l_sb, ht, xh_sb, xl_sb,
                                      skip_first=True, xl_pairs=3)
                            _s1_group(ps0, w0h_sb, w0l_sb, ht, xh_sb, xl_sb,
                                      skip_first=True)
                        else:
                            ps1 = pp1.tile([P, tcw], fp32, tag="ps1")
                            _s1_group(ps1, w1h_sb, w1l_sb, ht, xh_sb, xl_sb,
                                      xl_pairs=3)
                            ps0 = pp0.tile([P, tcw], fp32, tag="ps0")
                            _s1_group(ps0, w0h_sb, w0l_sb, ht, xh_sb, xl_sb)
                        sil = spool.tile([P, tcw], fp32, tag="sil")
                        af = AF.Silu if _ACT_SILU else AF.Tanh
                        # sil = silu(h1 + b1) with h1 = ps1/SW
                        nc.scalar.activation(
                            sil[:], ps1[:], af,
                            bias=b1t_sb[:, ht:ht + 1], scale=1.0 / SW,
                        )
                        # af32 = (ps0 + SW*b0) * sil = SW * act
                        last_pair = ht >= KH - 2
                        af32 = fpool.tile([P, tcw], fp32, tag="af32")
                        nc.vector.scalar_tensor_tensor(
                            af32[:], ps0[:], b0s_sb[:, ht:ht + 1], sil[:],
                            ALU.add, ALU.mult,
                        )
                        # act_hi = q8(SA * act), scale SA/SW
                        ah_t = ah_sb[ht // 2][:, ht % 2, :]
                        if last_pair:
                            # keep the whole chain on DVE: no cross-engine
                            # sem hops on the path stage 2's last pair waits on
                            nc.vector.tensor_scalar(
                                ah_t, af32[:], SA / SW, None, ALU.mult,
                            )
                        else:
                            nc.scalar.activation(
                                ah_t, af32[:], AF.Copy, scale=SA / SW,
                            )
                        # act_lo = q8(SA*act - act_hi)
                        nc.vector.scalar_tensor_tensor(
                            al_sb[ht // 2][:, ht % 2, :], af32[:], SA / SW,
                            ah_t, ALU.mult, ALU.subtract,
                        )
                    # stage 2 transposed: out_t[d-tile] = w2T_tile.T @ act
                    last_chunk = c0 + tcw == C
                    for dk in range(KD):
                        # split the very last group into token-halves so the
                        # first half's copy+DMA overlaps the second half's
                        # matmuls (shorter kernel tail)
                        if last_chunk and dk == KD - 1 and tcw >= 128:
                            halves = [(0, tcw - _LASTP), (tcw - _LASTP, _LASTP)]
                        else:
                            halves = [(0, tcw)]
                        ds = slice(dk * P, (dk + 1) * P)
                        for f0, fw in halves:
                            pso = ppo.tile([P, fw], fp32, tag="pso")
                            terms = [(w2h_sb, ah_sb), (w2l_sb, ah_sb),
                                     (w2h_sb, al_sb)]
                            # j-major so pair j's act tiles are needed as
                            # late as possible: the last h-pair's epilogue
                            # (still in flight right after stage 1) is only
                            # read by matmuls 21..23 of the group.
                            n = len(terms) * (KH // 2)
                            i = 0
                            for j in range(KH // 2):
                                for w_sb, a_sb in terms:
                                    nc.tensor.matmul(
                                        pso[:],
                                        w_sb[:, 2 * j:2 * j + 2, ds],
                                        a_sb[j][:, :, f0:f0 + fw],
                                        start=(i == 0),
                                        stop=(i == n - 1),
                                        perf_mode=DR,
                                    )
                                    i += 1
                            o_sb = opool.tile([P, fw], bf16, tag="osb")
                            # descale 1/(SW*SA) folded into the PSUM->SBUF copy
                            nc.scalar.activation(
                                o_sb[:], pso[:], AF.Copy, scale=1.0 / (SW * SA),
                            )
                            nc.sync.dma_start(
                                out_d[:, dk, c0 + f0:c0 + f0 + fw], o_sb[:]
                            )
                    c0 += tcw

            if repeat == 1:
                _body()
            else:
                # hardware loop: constant program size for any repeat count
                # (used only for wall-clock benchmarking of the body)
                with tc.For_i(0, repeat, 1):
                    _body()
    nc.compile()
    return nc


def _get_bass(C: int, repeat: int = 1):
    key = (C, repeat)
    if key not in _build_cache:
        _build_cache[key] = _build_bass(C, repeat)
    return _build_cache[key]


_runner_cache: dict = {}


def _get_runner(C: int, repeat: int = 1):
    """Compile the SPMD program once and return a reusable launcher.

    Mirrors concourse.bass2jax.run_bass_via_pjrt but memoizes the jitted
    executable so repeated kernel() calls don't recompile the NEFF.
    """
    key = (C, repeat)
    if key in _runner_cache:
        return _runner_cache[key]

    import jax
    from jax.experimental.shard_map import shard_map
    from jax.sharding import Mesh, PartitionSpec
    import concourse.mybir as mybir
    from concourse import bass2jax

    nc = _get_bass(C, repeat)
    bass2jax.install_neuronx_cc_hook()
    partition_name = nc.partition_id_tensor.name if nc.partition_id_tensor else None

    in_names: list = []
    out_names: list = []
    out_avals: list = []
    out_shapes: list = []
    for alloc in nc.m.functions[0].allocations:
        if not isinstance(alloc, mybir.MemoryLocationSet):
            continue
        name = alloc.memorylocations[0].name
        if alloc.kind == "ExternalInput":
            if name != partition_name:
                in_names.append(name)
        elif alloc.kind == "ExternalOutput":
            shape = tuple(alloc.tensor_shape)
            dtype = mybir.dt.np(alloc.dtype)
            out_names.append(name)
            out_avals.append(jax.core.ShapedArray(shape, dtype))
            out_shapes.append((shape, dtype))
    n_params = len(in_names)
    all_names = list(in_names) + list(out_names)
    if partition_name is not None:
        all_names.append(partition_name)
    donate = tuple(range(n_params, n_params + len(out_names)))

    def _body(*args):
        operands = list(args)
        if partition_name is not None:
            operands.append(bass2jax.partition_id_tensor())
        outs = bass2jax._bass_exec_p.bind(
            *operands,
            out_avals=tuple(out_avals),
            in_names=tuple(all_names),
            out_names=tuple(out_names),
            lowering_input_output_aliases=(),
            sim_require_finite=True,
            sim_require_nnan=True,
            nc=nc,
        )
        return tuple(outs)

    devices = jax.devices()[:NCORES]
    assert len(devices) == NCORES
    mesh = Mesh(np.asarray(devices), ("core",))
    in_specs = (PartitionSpec("core"),) * (n_params + len(out_names))
    out_specs = (PartitionSpec("core"),) * len(out_names)
    sharded = jax.jit(
        shard_map(
            _body, mesh=mesh, in_specs=in_specs, out_specs=out_specs, check_rep=False
        ),
        donate_argnums=donate,
        keep_unused=True,
    )

    def run(in_maps):
        concat_in = [
            np.concatenate([np.asarray(in_maps[c][nm]) for c in range(NCORES)], axis=0)
            for nm in in_names
        ]
        concat_zeros = [
            np.zeros((NCORES * s[0], *s[1:]), dt) for s, dt in out_shapes
        ]
        out_arrs = sharded(*concat_in, *concat_zeros)
        return [
            {
                nm: np.asarray(out_arrs[i]).reshape(NCORES, *out_shapes[i][0])[c]
                for i, nm in enumerate(out_names)
            }
            for c in range(NCORES)
        ]

    _runner_cache[key] = run
    return run


def _route(x2d: np.ndarray, gate_w: np.ndarray, gate_b: np.ndarray):
    """Top-2 routing on the host (f64 logits for stable ordering)."""
    lg = x2d.astype(np.float64) @ gate_w.astype(np.float64).T
    lg += gate_b.astype(np.float64)
    order = np.argsort(-lg, axis=1, kind="stable")
    ti = order[:, :TOPK]
    tv = np.take_along_axis(lg, ti, axis=1)
    m = tv.max(axis=1, keepdims=True)
    ew = np.exp(tv - m)
    wk = ew / ew.sum(axis=1, keepdims=True)
    return ti, wk


def _tile_kxm(a: np.ndarray, ktiles: int) -> np.ndarray:
    """[Kdim, M] -> [128, ktiles, M] with Kdim = ktiles*128 on partitions."""
    kdim, m = a.shape
    assert kdim == ktiles * P
    return np.ascontiguousarray(a.reshape(ktiles, P, m).transpose(1, 0, 2))


def _tile_w01(w: np.ndarray) -> np.ndarray:
    """[H, D] fp8 weight (pre-quantized) -> [128, KH, KD, 128] h-tile-major."""
    a = _tile_kxm(np.ascontiguousarray(w.T), KD)  # [P, KD, H]
    return np.ascontiguousarray(
        a.reshape(P, KD, KH, P).transpose(0, 2, 1, 3)
    )


def _hi_lo(a: np.ndarray, s: float):
    """fp32 array -> (q8(s*a), q8(s*a - hi)) both e4m3."""
    sa = (a * s).astype(np.float32)
    hi = sa.astype(F8)
    lo = (sa - hi.astype(np.float32)).astype(F8)
    return hi, lo


def _prepare(x, gate_w, gate_b, w0, b0, w1, b1, w2, b2):
    """Host-side routing + per-core input packing. Returns (in_maps, meta)."""
    x = np.asarray(x)
    gate_w = np.asarray(gate_w, dtype=np.float32)
    gate_b = np.asarray(gate_b, dtype=np.float32)
    w0 = np.asarray(w0, dtype=np.float32)
    b0 = np.asarray(b0, dtype=np.float32)
    w1 = np.asarray(w1, dtype=np.float32)
    b1 = np.asarray(b1, dtype=np.float32)
    w2 = np.asarray(w2, dtype=np.float32)
    b2 = np.asarray(b2, dtype=np.float32)

    Bn, Sq, Dv = x.shape
    T = Bn * Sq
    x2d = np.ascontiguousarray(x.reshape(T, Dv)).astype(np.float32, copy=False)

    ti, wk = _route(x2d, gate_w, gate_b)

    idxs, wgts = [], []
    for e in range(E):
        sel = [np.nonzero(ti[:, k] == e)[0] for k in range(TOPK)]
        idxs.append(np.concatenate(sel))
        wgts.append(np.concatenate([wk[s, k] for k, s in enumerate(sel)]))

    maxc = max(len(i) for i in idxs)
    C = max(P, maxc)

    in_maps = []
    for e in range(E):
        xg = np.zeros((C, Dv), dtype=np.float32)
        xg[: len(idxs[e])] = x2d[idxs[e]]
        xh, xl = _hi_lo(xg.T, 1.0)
        w0h, w0l = _hi_lo(w0[e], SW)
        w1h, w1l = _hi_lo(w1[e], SW)
        w2h, w2l = _hi_lo(w2[e], SW)
        in_maps.append(
            {
                "xth": _tile_kxm(np.ascontiguousarray(xh), KD),
                "xtl": _tile_kxm(np.ascontiguousarray(xl), KD),
                "w0h": _tile_w01(w0h),
                "w0l": _tile_w01(w0l),
                "w1h": _tile_w01(w1h),
                "w1l": _tile_w01(w1l),
                "w2h": _tile_kxm(np.ascontiguousarray(w2h.T), KH),
                "w2l": _tile_kxm(np.ascontiguousarray(w2l.T), KH),
                "b0s": np.ascontiguousarray((b0[e] * SW).reshape(KH, P).T),
                "b1t": np.ascontiguousarray(b1[e].reshape(KH, P).T),
            }
        )
    meta = (Bn, Sq, Dv, T, C, idxs, wgts, b2)
    return in_maps, meta


def _combine(results, meta):
    Bn, Sq, Dv, T, C, idxs, wgts, b2 = meta
    out = np.zeros((T, Dv), dtype=np.float32)
    for e in range(E):
        n = len(idxs[e])
        # out_t [128, KD, C] -> [C, D] with d = k*128 + p
        ot = np.asarray(results[e]["out"]).astype(np.float32)
        o = ot.transpose(2, 1, 0).reshape(C, Dv)[:n]
        out[idxs[e]] += wgts[e][:, None].astype(np.float32) * (o + b2[e][None, :])
    return out.reshape(Bn, Sq, Dv)


def kernel(x, gate_w, gate_b, w0, b0, w1, b1, w2, b2):
    in_maps, meta = _prepare(x, gate_w, gate_b, w0, b0, w1, b1, w2, b2)
    C = meta[4]
    results = None
    last_err = None
    # The tunneled cores occasionally wedge mid-run; plain retries, then
    # retries that rebuild the PJRT client + executable from scratch.
    for attempt, (delay, rebuild) in enumerate(
        [(0.0, False), (2.0, False), (10.0, True), (20.0, True)]
    ):
        import time as _time

        if delay:
            _time.sleep(delay)
        if rebuild:
            import jax

            _runner_cache.clear()
            try:
                jax.clear_caches()
                jax.extend.backend.clear_backends()
            except Exception:
                pass
        try:
            results = _get_runner(C)(in_maps)
            break
        except Exception as e:  # noqa: BLE001
            last_err = e
    if results is None:
        raise last_err
    return _combine(results, meta)
